# revision 1
# baseline (speedup 1.0000x reference)
"""GQA forward (b=2, s=2048, H=32 q heads, 8 kv heads, d=64) on 8 TRN2 cores.

Sharding: core k owns query heads 4k..4k+3 and kv head k. GQA group
structure makes attention fully local per core (q heads 4k..4k+3 attend
only to kv head k). x is replicated; W columns are sharded; outputs are
column-concatenated.

Per-core kernel (Tile framework):
  - x.T tiles produced via PE transpose, evicted by DVE.
  - Projections in natural layout: QKV[s,384] = xT_chunk.T @ W_chunk
    accumulated over 16 k-chunks (x.T stationary, W moving).
  - RoPE on DVE with free-dim stride-2 views (partition strides are
    illegal), fused with the PSUM->SBUF eviction; V columns pass through
    straight into the [V|1] resident (already [kv, d] natural layout).
  - Q/K flipped to [d, s] via PE transposes.
  - Attention in transposed layout: S.T[kv,q] = K @ Q.T per 128-kv block,
    exp on ACT (scale=1/8 folded in), causal handled by skipping blocks
    above the diagonal + triangular predicated masks on diagonal blocks,
    ctx.T[65,q] = [V|1].T @ P.T accumulated in PSUM (row 64 = softmax sums).
  - Finalize: PE-transpose ctx.T back to [q,d], normalize by row sums, DMA.
Matmuls run as float32r (fp32 storage, full-rate PE mode).
"""

import numpy as np
from contextlib import ExitStack

import concourse.bass as bass
import concourse.bacc as bacc
import concourse.mybir as mybir
from concourse import tile
from concourse.bass_utils import run_bass_kernel_spmd

F32 = mybir.dt.float32
F32R = mybir.dt.float32r
U8 = mybir.dt.uint8
MUL = mybir.AluOpType.mult
ADD = mybir.AluOpType.add

B = 2
S = 2048
DIN = 2048
D = 64              # head dim
HPC = 4             # query heads per core
NCORES = 8
WCOLS = 4 * D + D + D  # 256 q cols + 64 k + 64 v = 384
RC = 320            # roped columns (4 q heads + k head)
ST = 512            # s-tile (rows per outer step)
NST = B * S // ST   # 8 s-tiles
NCH = DIN // 128    # 16 k-chunks
NKV = S // 128      # kv tiles per batch
NEG = -30000.0      # pre-scale mask fill; exp(NEG/8) == 0 in f32


def build_bass():
    nc = bacc.Bacc(None, target_bir_lowering=False)
    x_d = nc.declare_dram_parameter("x", [B * S, DIN], F32, isOutput=False)
    w_d = nc.declare_dram_parameter("w", [DIN, WCOLS], F32, isOutput=False)
    cos_d = nc.declare_dram_parameter("cosn", [S, RC], F32, isOutput=False)
    sin_d = nc.declare_dram_parameter("sinn", [S, RC], F32, isOutput=False)
    mask_d = nc.declare_dram_parameter("mask", [128, 128], U8, isOutput=False)
    id_d = nc.declare_dram_parameter("ident", [128, 128], F32, isOutput=False)
    out_d = nc.declare_dram_parameter("out", [B * S, HPC * D], F32, isOutput=True)

    with ExitStack() as ctx:
        tc = ctx.enter_context(tile.TileContext(nc))
        const = ctx.enter_context(tc.tile_pool(name="const", bufs=1))
        resid = ctx.enter_context(tc.tile_pool(name="resid", bufs=1))
        xa_p = ctx.enter_context(tc.tile_pool(name="xa", bufs=2))
        xt_p = ctx.enter_context(tc.tile_pool(name="xt", bufs=2))
        tab_p = ctx.enter_context(tc.tile_pool(name="tab", bufs=3))
        qn_p = ctx.enter_context(tc.tile_pool(name="qn", bufs=3))
        qt_p = ctx.enter_context(tc.tile_pool(name="qt", bufs=4))
        p_p = ctx.enter_context(tc.tile_pool(name="p", bufs=3))
        cx_p = ctx.enter_context(tc.tile_pool(name="cx", bufs=2))
        o_p = ctx.enter_context(tc.tile_pool(name="o", bufs=3))
        rv_p = ctx.enter_context(tc.tile_pool(name="rv", bufs=4))
        tp_ps = ctx.enter_context(tc.tile_pool(name="tp_ps", bufs=2, space="PSUM"))
        pr_ps = ctx.enter_context(tc.tile_pool(name="pr_ps", bufs=2, space="PSUM"))
        sc_ps = ctx.enter_context(tc.tile_pool(name="sc_ps", bufs=2, space="PSUM"))
        cx_ps = ctx.enter_context(tc.tile_pool(name="cx_ps", bufs=1, space="PSUM"))
        fi_ps = ctx.enter_context(tc.tile_pool(name="fi_ps", bufs=1, space="PSUM"))

        # constants
        w_sb = const.tile([128, NCH, WCOLS], F32R)
        nc.sync.dma_start(
            out=w_sb[:],
            in_=w_d.rearrange("(c p) n -> p c n", p=128).bitcast(F32R))
        mask_sb = const.tile([128, 128], U8)
        nc.sync.dma_start(out=mask_sb[:], in_=mask_d[:])
        ident = const.tile([128, 128], F32R)
        nc.sync.dma_start(out=ident[:], in_=id_d[:].bitcast(F32R))
        neg_sb = const.tile([128, 128], F32)
        nc.vector.memset(neg_sb[:], NEG)
        one_sb = const.tile([128, 1], F32)
        nc.vector.memset(one_sb[:], 1.0)
        zero_sb = const.tile([128, 63], F32)
        nc.vector.memset(zero_sb[:], 0.0)

        # rows 0-63: K.T (RoPE'd); rows 64-127: duplicate copy so that the
        # scores matmul lhsT can match either base partition of the Q halves
        kt_res = resid.tile([128, B * S], F32R)
        vp_res = resid.tile([128, B * NKV, 128], F32R)  # [V|1|0pad] kv-tiles
        for slot in range(B * NKV):
            nc.vector.tensor_copy(vp_res[:, slot, 64:65], one_sb[:])
            nc.vector.tensor_copy(vp_res[:, slot, 65:128], zero_sb[:])

        for st in range(NST):
            b, sti = divmod(st, 4)

            # ---- x rows -> x.T tiles ----
            xt = xt_p.tile([128, NCH, ST], F32R)
            for pt in range(4):
                xa = xa_p.tile([128, DIN], F32R, tag="xa")
                nc.sync.dma_start(
                    out=xa[:],
                    in_=x_d[st * ST + pt * 128:
                            st * ST + (pt + 1) * 128, :].bitcast(F32R))
                for c in range(NCH):
                    tp = tp_ps.tile([128, 128], F32R, tag="tp")
                    nc.tensor.transpose(tp[:], xa[:, c * 128:(c + 1) * 128],
                                        ident[:])
                    nc.vector.tensor_copy(
                        xt[:, c, pt * 128:(pt + 1) * 128], tp[:])

            # ---- projections (natural layout) + RoPE + transposes ----
            qta = qt_p.tile([128, ST], F32R, tag="qta")   # heads 0,1 as [d,s]
            qtb = qt_p.tile([128, ST], F32R, tag="qtb")   # heads 2,3 as [d,s]
            for pt in range(4):
                t = sti * 4 + pt  # within-batch 128-row block index
                pp = pr_ps.tile([128, WCOLS], F32, tag="pp")
                for c in range(NCH):
                    nc.tensor.matmul(
                        pp[:], xt[:, c, pt * 128:(pt + 1) * 128],
                        w_sb[:, c, :], start=(c == 0), stop=(c == NCH - 1))
                ctab = tab_p.tile([128, RC], F32, tag="ctab")
                nc.sync.dma_start(out=ctab[:],
                                  in_=cos_d[t * 128:(t + 1) * 128, :])
                stab = tab_p.tile([128, RC], F32, tag="stab")
                nc.sync.dma_start(out=stab[:],
                                  in_=sin_d[t * 128:(t + 1) * 128, :])
                qn = qn_p.tile([128, WCOLS], F32R, tag="qn")
                ts = qn_p.tile([128, RC], F32, tag="ts")
                # even cols: qe*c - qo*s ; odd cols: qo*c + qe*s
                nc.vector.scalar_tensor_tensor(
                    ts[:, 0:RC:2], pp[:, 1:RC:2], -1.0, stab[:, 0:RC:2],
                    MUL, MUL)
                nc.vector.tensor_tensor(
                    ts[:, 1:RC:2], pp[:, 0:RC:2], stab[:, 1:RC:2], MUL)
                nc.vector.tensor_tensor(qn[:, 0:RC], pp[:, 0:RC], ctab[:], MUL)
                nc.vector.tensor_tensor(qn[:, 0:RC], qn[:, 0:RC], ts[:], ADD)
                # V columns: straight into the [V|1] resident (natural [kv,d])
                nc.vector.tensor_copy(
                    vp_res[:, b * NKV + t, 0:64], pp[:, RC:WCOLS])
                # also land V in qn so the padded K-flip transpose below reads
                # initialized data (its V rows are discarded)
                nc.vector.tensor_copy(qn[:, RC:WCOLS], pp[:, RC:WCOLS])
                # flip Q/K to [d, s]
                for cb in range(2):
                    tp = tp_ps.tile([128, 128], F32R, tag="tp")
                    nc.tensor.transpose(
                        tp[:], qn[:, cb * 128:(cb + 1) * 128], ident[:])
                    dst = qta if cb == 0 else qtb
                    nc.vector.tensor_copy(
                        dst[:, pt * 128:(pt + 1) * 128], tp[:])
                tp = tp_ps.tile([128, 128], F32R, tag="tp")
                nc.tensor.transpose(tp[:], qn[:, 256:384], ident[:])
                nc.vector.tensor_copy(
                    kt_res[0:64, st * ST + pt * 128:st * ST + (pt + 1) * 128],
                    tp[0:64, :])
            nc.sync.dma_start(
                out=kt_res[64:128, st * ST:(st + 1) * ST],
                in_=kt_res[0:64, st * ST:(st + 1) * ST])

            # ---- attention for the 4 heads of this q-tile ----
            js = [4 * sti] + list(range(4 * sti)) + \
                 [4 * sti + 1, 4 * sti + 2, 4 * sti + 3]
            for h in range(HPC):
                p0 = (h % 2) * 64
                qh = (qta if h < 2 else qtb)[p0:p0 + 64, :]
                cxt = cx_ps.tile([128, ST], F32, tag="cxt")
                for idx, j in enumerate(js):
                    off = 128 * j - 512 * sti
                    if j <= 4 * sti:
                        w0 = 0
                    elif off == 128:
                        w0 = 128
                    else:
                        w0 = 256
                    sc = sc_ps.tile([128, ST], F32, tag="sc")
                    nc.tensor.matmul(
                        sc[:, w0:ST],
                        kt_res[p0:p0 + 64, b * S + j * 128:b * S + (j + 1) * 128],
                        qh[:, w0:ST], start=True, stop=True)
                    if j == 4 * sti + 3:
                        nc.vector.tensor_copy(sc[:, 256:384], neg_sb[:])
                    if j >= 4 * sti:
                        nc.vector.copy_predicated(
                            sc[:, off:off + 128], mask_sb[:], neg_sb[:])
                    psb = p_p.tile([128, ST], F32R, tag="psb")
                    nc.scalar.activation(
                        psb[:, w0:ST], sc[:, w0:ST],
                        mybir.ActivationFunctionType.Exp, scale=0.125)
                    nc.tensor.matmul(
                        cxt[:, w0:ST], vp_res[:, b * NKV + j, :],
                        psb[:, w0:ST],
                        start=(idx == 0), stop=(idx == len(js) - 1))
                cxs = cx_p.tile([65, ST], F32R, tag="cxs")
                nc.vector.tensor_copy(cxs[:], cxt[0:65, :])
                for qq in range(4):
                    fi = fi_ps.tile([128, 66], F32R, tag="fi")
                    nc.tensor.transpose(fi[:], cxs[:, qq * 128:(qq + 1) * 128],
                                        ident[0:65, 0:66])
                    rv = rv_p.tile([128, 1], F32, tag="rv")
                    nc.vector.reciprocal(rv[:], fi[:, 64:65])
                    ob = o_p.tile([128, 64], F32, tag="ob")
                    nc.vector.tensor_scalar_mul(ob[:], fi[:, 0:64], rv[:])
                    nc.sync.dma_start(
                        out=out_d[st * ST + qq * 128:st * ST + (qq + 1) * 128,
                                  h * 64:(h + 1) * 64],
                        in_=ob[:])
    return nc


_NC_CACHE = None


def _host_consts():
    i = np.arange(0, D, 2, dtype=np.float64) / D          # 32 pair exponents
    freqs = 1.0 / (10000.0 ** i)                           # (32,)
    ang = np.arange(S, dtype=np.float64)[:, None] * freqs[None, :]  # (S, 32)
    cos = np.cos(ang).astype(np.float32)                   # (S, 32)
    sin = np.sin(ang).astype(np.float32)
    dcol = (np.arange(RC) % D) // 2                        # (320,) pair idx
    cosn = np.ascontiguousarray(cos[:, dcol])              # (S, 320)
    sinn = np.ascontiguousarray(sin[:, dcol])
    kv, qq = np.meshgrid(np.arange(128), np.arange(128), indexing="ij")
    maskinv = (kv > qq).astype(np.uint8)                   # 1 = forbidden
    ident = np.eye(128, dtype=np.float32)
    return cosn, sinn, maskinv, ident


def _in_maps(x, Wq, Wk, Wv):
    x = np.asarray(x, dtype=np.float32).reshape(B * S, DIN)
    Wq = np.asarray(Wq, dtype=np.float32)
    Wk = np.asarray(Wk, dtype=np.float32)
    Wv = np.asarray(Wv, dtype=np.float32)
    cosn, sinn, maskinv, ident = _host_consts()

    in_maps = []
    for k in range(NCORES):
        w_all = np.hstack([
            Wq[:, k * 256:(k + 1) * 256],
            Wk[:, k * 64:(k + 1) * 64],
            Wv[:, k * 64:(k + 1) * 64],
        ]).astype(np.float32)
        in_maps.append({
            "x": x, "w": np.ascontiguousarray(w_all),
            "cosn": cosn, "sinn": sinn, "mask": maskinv, "ident": ident,
        })
    return in_maps


def _run(in_maps, **kwargs):
    global _NC_CACHE
    if _NC_CACHE is None:
        _NC_CACHE = build_bass()
        _NC_CACHE.finalize()
    return run_bass_kernel_spmd(_NC_CACHE, in_maps, list(range(NCORES)),
                                **kwargs)


def kernel(x, Wq, Wk, Wv):
    res = _run(_in_maps(x, Wq, Wk, Wv))
    out = np.concatenate([res.results[k]["out"] for k in range(NCORES)], axis=1)
    return out.reshape(B, S, 32 * D)



# revision 10
# speedup vs baseline: 2.0981x; 2.0981x over previous
"""GQA forward (b=2, s=2048, H=32 q heads, 8 kv heads, d=64) on 8 TRN2 cores.

Sharding: core k owns query heads 4k..4k+3 and kv head k. GQA group
structure makes attention fully local per core (q heads 4k..4k+3 attend
only to kv head k). x is replicated; W columns are sharded; outputs are
column-concatenated.

Per-core kernel (Tile framework), v2 — bf16 + transposed dataflow:
  - Host passes x.T (pre-transposed, bf16) so no PE transposes are spent
    producing x.T tiles; W is passed bf16 with head-dim columns
    de-interleaved (evens then odds per head) so RoPE works on
    contiguous partition blocks in the transposed projection layout.
  - Projections directly in transposed layout: QKV.T[cols,s] tiles =
    W_chunk.T @ xT_chunk accumulated over 16 k-chunks (W stationary).
    Col-blocks: [q0|q1], [q2|q3], [k|v], each row-packed [e32 o32] per
    head.
  - RoPE fused with PSUM eviction on DVE: 32-row partition-block ops,
    f32 intermediates, bf16 results straight into Q tiles / K resident.
  - V.T rows are flipped to natural [kv, d] via 4 small PE transposes
    per s-tile into the [V|1] resident.
  - Attention in transposed layout (bf16): S.T[kv,q] = K @ Q.T per
    128-kv block, exp on ACT (scale=1/8 folded), causal via block
    skipping + triangular predicated masks on diagonal blocks,
    ctx.T[65,q] = [V|1].T @ P.T accumulated in PSUM (row 64 = sums).
  - Finalize: PE-transpose ctx.T back to [q,d], normalize by row sums
    (f32), DMA out f32.
"""

import numpy as np
from contextlib import ExitStack

import ml_dtypes

import concourse.bass as bass
import concourse.bacc as bacc
import concourse.mybir as mybir
from concourse import tile
from concourse.bass_utils import run_bass_kernel_spmd

F32 = mybir.dt.float32
F32R = mybir.dt.float32r
BF16 = mybir.dt.bfloat16
U8 = mybir.dt.uint8
BF16NP = ml_dtypes.bfloat16
MUL = mybir.AluOpType.mult
ADD = mybir.AluOpType.add

B = 2
S = 2048
DIN = 2048
D = 64              # head dim
HPC = 4             # query heads per core
NCORES = 8
WCOLS = 384         # 3 col-blocks of 128: [q0|q1], [q2|q3], [k|v]
ST = 512            # s-tile (rows per outer step)
NST = B * S // ST   # 8 s-tiles
NCH = DIN // 128    # 16 k-chunks
NKV = S // 128      # kv tiles per batch
NEG = -30000.0      # pre-scale mask fill; exp(NEG/8) == 0 in f32


def build_bass():
    nc = bacc.Bacc(None, target_bir_lowering=False)
    xt_d = nc.declare_dram_parameter("xt", [DIN, B * S], BF16, isOutput=False)
    w_d = nc.declare_dram_parameter("w", [DIN, WCOLS], BF16, isOutput=False)
    cos_d = nc.declare_dram_parameter("cosq", [128, S], BF16, isOutput=False)
    sin_d = nc.declare_dram_parameter("sinq", [128, S], BF16, isOutput=False)
    mask_d = nc.declare_dram_parameter("mask", [128, 128], U8, isOutput=False)
    id_d = nc.declare_dram_parameter("ident", [128, 128], BF16, isOutput=False)
    idf_d = nc.declare_dram_parameter("identf", [128, 128], F32, isOutput=False)
    out_d = nc.declare_dram_parameter("out", [B * S, HPC * D], F32, isOutput=True)

    with ExitStack() as ctx:
        tc = ctx.enter_context(tile.TileContext(nc))
        const = ctx.enter_context(tc.tile_pool(name="const", bufs=1))
        resid = ctx.enter_context(tc.tile_pool(name="resid", bufs=1))
        xa_p = ctx.enter_context(tc.tile_pool(name="xa", bufs=2))
        qt_p = ctx.enter_context(tc.tile_pool(name="qt", bufs=4))
        tmp_p = ctx.enter_context(tc.tile_pool(name="tmp", bufs=3))
        vt_p = ctx.enter_context(tc.tile_pool(name="vt", bufs=2))
        p_p = ctx.enter_context(tc.tile_pool(name="p", bufs=3))
        cx_p = ctx.enter_context(tc.tile_pool(name="cx", bufs=2))
        o_p = ctx.enter_context(tc.tile_pool(name="o", bufs=3))
        rv_p = ctx.enter_context(tc.tile_pool(name="rv", bufs=4))
        tp_ps = ctx.enter_context(tc.tile_pool(name="tp_ps", bufs=2, space="PSUM"))
        pr_ps = ctx.enter_context(tc.tile_pool(name="pr_ps", bufs=2, space="PSUM"))
        sc_ps = ctx.enter_context(tc.tile_pool(name="sc_ps", bufs=2, space="PSUM"))
        cx_ps = ctx.enter_context(tc.tile_pool(name="cx_ps", bufs=1, space="PSUM"))
        fi_ps = ctx.enter_context(tc.tile_pool(name="fi_ps", bufs=1, space="PSUM"))

        # constants
        w_sb = const.tile([128, NCH, WCOLS], BF16)
        nc.sync.dma_start(
            out=w_sb[:], in_=w_d.rearrange("(c p) n -> p c n", p=128))
        mask_sb = const.tile([128, 128], U8)
        nc.sync.dma_start(out=mask_sb[:], in_=mask_d[:])
        ident = const.tile([128, 128], BF16)
        nc.sync.dma_start(out=ident[:], in_=id_d[:])
        identf = const.tile([128, 128], F32R)
        nc.sync.dma_start(out=identf[:], in_=idf_d[:].bitcast(F32R))
        cos_sb = const.tile([128, S], BF16)
        nc.sync.dma_start(out=cos_sb[:], in_=cos_d[:])
        sin_sb = const.tile([128, S], BF16)
        nc.sync.dma_start(out=sin_sb[:], in_=sin_d[:])
        neg_sb = const.tile([128, 128], F32)
        nc.vector.memset(neg_sb[:], NEG)

        # K.T resident (RoPE'd, bf16); rows 64-127 duplicate rows 0-63 so
        # the scores lhsT can match either base partition of the Q halves.
        kt_res = resid.tile([128, B * S], BF16)
        # [V | 1 | pad] kv-tiles, natural [kv, d] layout
        vp_res = resid.tile([128, B * NKV, 66], BF16)
        nc.vector.memset(vp_res[:, :, 64:65], 1.0)

        xt_dram = xt_d.rearrange("(c p) s -> p c s", p=128)

        for st in range(NST):
            b, sti = divmod(st, 4)
            scol = slice(sti * ST, (sti + 1) * ST)  # within-batch position

            # ---- x.T tile straight from DRAM ----
            xa = xa_p.tile([128, NCH, ST], BF16, tag="xa")
            nc.sync.dma_start(out=xa[:], in_=xt_dram[:, :, st * ST:(st + 1) * ST])

            # ---- projections (transposed layout) + RoPE ----
            qa = qt_p.tile([128, ST], BF16, tag="qa")   # [q0_e q0_o q1_e q1_o]
            qb = qt_p.tile([128, ST], BF16, tag="qb")   # [q2_e q2_o q3_e q3_o]
            for cb in range(3):
                pp = pr_ps.tile([128, ST], F32, tag="pp")
                for c in range(NCH):
                    nc.tensor.matmul(
                        pp[:], w_sb[:, c, cb * 128:(cb + 1) * 128], xa[:, c, :],
                        start=(c == 0), stop=(c == NCH - 1))
                ts = tmp_p.tile([128, ST], F32, tag="ts")
                qn = tmp_p.tile([128, ST], F32, tag="qn")
                if cb < 2:
                    dst = qa if cb == 0 else qb
                    # rows per head h (0/1): [e at 64h..64h+32, o at +32..+64]
                    for hh in range(2):
                        r = 64 * hh
                        nc.vector.scalar_tensor_tensor(
                            ts[r:r + 32, :], pp[r + 32:r + 64, :], -1.0,
                            sin_sb[r:r + 32, scol], MUL, MUL)
                        nc.vector.tensor_tensor(
                            ts[r + 32:r + 64, :], pp[r:r + 32, :],
                            sin_sb[r + 32:r + 64, scol], MUL)
                    nc.vector.tensor_tensor(qn[:], pp[:], cos_sb[:, scol], MUL)
                    nc.vector.tensor_tensor(dst[:], qn[:], ts[:], ADD)
                else:
                    # rows: [k_e(32) k_o(32) | v(64)]
                    nc.vector.scalar_tensor_tensor(
                        ts[0:32, :], pp[32:64, :], -1.0,
                        sin_sb[0:32, scol], MUL, MUL)
                    nc.vector.tensor_tensor(
                        ts[32:64, :], pp[0:32, :], sin_sb[32:64, scol], MUL)
                    nc.vector.tensor_tensor(
                        qn[0:64, :], pp[0:64, :], cos_sb[0:64, scol], MUL)
                    nc.vector.tensor_tensor(
                        kt_res[0:64, st * ST:(st + 1) * ST],
                        qn[0:64, :], ts[0:64, :], ADD)
                    vt = vt_p.tile([64, ST], BF16, tag="vt")
                    nc.vector.tensor_copy(vt[:], pp[64:128, :])
                    for kb in range(4):
                        tp = tp_ps.tile([128, 64], BF16, tag="tp")
                        nc.tensor.transpose(
                            tp[:], vt[:, kb * 128:(kb + 1) * 128],
                            ident[0:64, 0:64])
                        nc.vector.tensor_copy(
                            vp_res[:, b * NKV + sti * 4 + kb, 0:64], tp[:])
            nc.sync.dma_start(
                out=kt_res[64:128, st * ST:(st + 1) * ST],
                in_=kt_res[0:64, st * ST:(st + 1) * ST])

            # ---- attention for the 4 heads of this q-tile ----
            js = [4 * sti] + list(range(4 * sti)) + \
                 [4 * sti + 1, 4 * sti + 2, 4 * sti + 3]
            for h in range(HPC):
                p0 = (h % 2) * 64
                qh = (qa if h < 2 else qb)[p0:p0 + 64, :]
                cxt = cx_ps.tile([65, ST], F32, tag="cxt")
                for idx, j in enumerate(js):
                    off = 128 * j - 512 * sti
                    if j <= 4 * sti:
                        w0 = 0
                    elif off == 128:
                        w0 = 128
                    else:
                        w0 = 256
                    sc = sc_ps.tile([128, ST], F32, tag="sc")
                    nc.tensor.matmul(
                        sc[:, w0:ST],
                        kt_res[p0:p0 + 64, b * S + j * 128:b * S + (j + 1) * 128],
                        qh[:, w0:ST], start=True, stop=True)
                    if j == 4 * sti + 3:
                        nc.vector.tensor_copy(sc[:, 256:384], neg_sb[:])
                    if j >= 4 * sti:
                        nc.vector.copy_predicated(
                            sc[:, off:off + 128], mask_sb[:], neg_sb[:])
                    psb = p_p.tile([128, ST], BF16, tag="psb")
                    nc.scalar.activation(
                        psb[:, w0:ST], sc[:, w0:ST],
                        mybir.ActivationFunctionType.Exp, scale=0.125)
                    nc.tensor.matmul(
                        cxt[:, w0:ST], vp_res[:, b * NKV + j, 0:65],
                        psb[:, w0:ST],
                        start=(idx == 0), stop=(idx == len(js) - 1))
                cxs = cx_p.tile([65, ST], F32R, tag="cxs")
                nc.vector.tensor_copy(cxs[:], cxt[:])
                for qq in range(4):
                    fi = fi_ps.tile([128, 66], F32R, tag="fi")
                    nc.tensor.transpose(fi[:], cxs[:, qq * 128:(qq + 1) * 128],
                                        identf[0:65, 0:66])
                    rv = rv_p.tile([128, 1], F32, tag="rv")
                    nc.vector.reciprocal(rv[:], fi[:, 64:65])
                    ob = o_p.tile([128, 64], F32, tag="ob")
                    nc.vector.tensor_scalar_mul(ob[:], fi[:, 0:64], rv[:])
                    nc.sync.dma_start(
                        out=out_d[st * ST + qq * 128:st * ST + (qq + 1) * 128,
                                  h * 64:(h + 1) * 64],
                        in_=ob[:])
    return nc


_NC_CACHE = None


def _host_consts():
    i = np.arange(0, D, 2, dtype=np.float64) / D          # 32 pair exponents
    freqs = 1.0 / (10000.0 ** i)                           # (32,)
    ang = np.arange(S, dtype=np.float64)[:, None] * freqs[None, :]  # (S, 32)
    cos32 = np.cos(ang).astype(np.float32).T               # (32, S)
    sin32 = np.sin(ang).astype(np.float32).T
    cosq = np.tile(cos32, (4, 1)).astype(BF16NP)           # (128, S)
    sinq = np.tile(sin32, (4, 1)).astype(BF16NP)
    kv, qq = np.meshgrid(np.arange(128), np.arange(128), indexing="ij")
    maskinv = (kv > qq).astype(np.uint8)                   # 1 = forbidden
    identf = np.eye(128, dtype=np.float32)
    return cosq, sinq, maskinv, identf.astype(BF16NP), identf


def _deint(w):
    # (din, 64) head cols -> [evens(32) | odds(32)]
    return np.hstack([w[:, 0::2], w[:, 1::2]])


def _in_maps(x, Wq, Wk, Wv):
    x = np.asarray(x, dtype=np.float32).reshape(B * S, DIN)
    xt = np.ascontiguousarray(x.T.astype(BF16NP))
    Wq = np.asarray(Wq, dtype=np.float32)
    Wk = np.asarray(Wk, dtype=np.float32)
    Wv = np.asarray(Wv, dtype=np.float32)
    cosq, sinq, maskinv, ident, identf = _host_consts()

    in_maps = []
    for k in range(NCORES):
        blocks = []
        for h in range(4):
            blocks.append(_deint(Wq[:, (4 * k + h) * 64:(4 * k + h + 1) * 64]))
        blocks.append(_deint(Wk[:, k * 64:(k + 1) * 64]))
        blocks.append(Wv[:, k * 64:(k + 1) * 64])
        w_all = np.hstack(blocks).astype(BF16NP)
        in_maps.append({
            "xt": xt, "w": np.ascontiguousarray(w_all),
            "cosq": cosq, "sinq": sinq, "mask": maskinv, "ident": ident,
            "identf": identf,
        })
    return in_maps


def _run(in_maps, **kwargs):
    global _NC_CACHE
    if _NC_CACHE is None:
        _NC_CACHE = build_bass()
        _NC_CACHE.finalize()
    return run_bass_kernel_spmd(_NC_CACHE, in_maps, list(range(NCORES)),
                                **kwargs)


def kernel(x, Wq, Wk, Wv):
    res = _run(_in_maps(x, Wq, Wk, Wv))
    out = np.concatenate([res.results[k]["out"] for k in range(NCORES)], axis=1)
    return out.reshape(B, S, 32 * D)
